# revision 40
# baseline (speedup 1.0000x reference)
"""MLA attention kernel for Trainium2, 8 NeuronCores (v2).

Sharding: core = (batch b in {0,1}) x (head-group hg in {0..3}, 4 heads each).

Front phase: the 4 cores of a batch group compute 1/4-sharded down-
projections (c_kv slice 128 rows + q_lat slice 192 rows each, SCALE folded
into q_lat) per 512-token column block for blocks 1-3, all-gathered via two
CC ops (block 1; blocks 2+3) through DRAM bounce buffers. Both CCs are
triggered as early as possible (the CC engine's bootstrap barrier takes
~65us, and the two gathers run back-to-back after it); the replicated
block-0 down-projections then fill the whole gather window with
CC-independent PE work, so U2(0)/U1(0)/att(0) never wait on a collective.
Gather reloads ride the gpsimd queue, parked on the CC semaphores without
blocking the sync/scalar load streams.

Middle is a per-q-block pipeline: U2(j) (k-rope first, then k-ups + V) ->
U1(j) -> deferred softmax tail of j-1 -> o_proj(j-1) -> att(j). Score
matmuls stream through a flat cursor 2 tiles ahead of consumption; masked
diagonal tiles are interleaved with non-diagonal ones for pipeline slack;
each head's den matmul is deferred into the next head's score stream.

dtypes: everything on the matmul path is bf16 (PSUM accumulates fp32);
partial outputs are written bf16 and summed on the host in fp32 (measured
rel err ~4.4e-3 vs the fp32 reference, gate 2e-2). Softmax: exp on ACT;
causal masking by multiplying the probs with precomputed 0/1 bf16 masks
(DVE; tensor_paged_mask and gpsimd affine_select on bf16 both crash this
HW); denominator via pairwise-tree bf16 adds on DVE + one all-ones [128,128]
matmul per head whose output replicates den across all partitions (kills
the partition_broadcast), then reciprocal_approx_fast. Weights are
host-packed into a handful of wide [128, N] tensors so the whole weight
load is ~20 large DMAs spread across the sync/scalar/gpsimd queues in
need-order.
"""
import sys

sys.path.insert(0, "/opt/trn_rl_repo")

import numpy as np
import ml_dtypes
import concourse.bass as bass
import concourse.bacc as bacc
import concourse.tile as tile
from concourse import mybir
from concourse.bass_utils import run_bass_kernel_spmd

FP = mybir.dt.float32
BF = mybir.dt.bfloat16
F16 = mybir.dt.float16
S = 2048
HID = 2048
H = 16
DN = 128
DR = 64
DV = 128
QL = 768
KVL = 512
ROPE_BASE = 10000.0
SCALE = (DN + DR) ** -0.5
NEG = -1e9
NCORES = 8
HPC = 4  # heads per core
P = 128
NB = S // 512  # 4 query/key column blocks of 512
KT = S // P  # 16 token tiles of 128
QLT = QL // P  # 6
KVT = KVL // P  # 4
QSH = QL // 4  # 192: per-core q_lat shard
SH = P + QSH  # 320: stacked ckv+ql shard rows

_cache = {}


def _build_v2(variant):
    """variant: 'causal' or 'zeros'."""
    nc = bacc.Bacc()
    mm = nc.tensor.matmul

    hidT = nc.dram_tensor("hidT", [HID, S], BF, kind="ExternalInput")
    # host-packed weights (all [128, N]; k-tiles along columns)
    wds_d = nc.dram_tensor("wds", [P, KT * SH], BF, kind="ExternalInput")
    wkf_d = nc.dram_tensor("wkf", [P, KT * KVL], BF, kind="ExternalInput")
    wqf_d = nc.dram_tensor("wqf", [P, KT * QL], BF, kind="ExternalInput")
    wuk_d = nc.dram_tensor("wuk", [P, KVT * (DN + DR + DV) * HPC], BF,
                           kind="ExternalInput")
    wuq_d = nc.dram_tensor("wuq", [P, QLT * (DN + DR) * HPC], BF,
                           kind="ExternalInput")
    wo_d = nc.dram_tensor("wop", [P, HPC * HID], BF, kind="ExternalInput")
    cs_d = nc.dram_tensor("cs", [P, 2 * S], BF, kind="ExternalInput")
    if variant == "causal":
        m01_d = nc.dram_tensor("m01", [P, 4 * 512], BF, kind="ExternalInput")
    o_out = nc.dram_tensor("o", [S, HID], BF, kind="ExternalOutput")

    REP = [[0, 1, 2, 3], [4, 5, 6, 7]]

    def nkt_of(jj):
        return 4 * (jj + 1) if variant == "causal" else KT

    with tile.TileContext(nc) as tc:
        # ---------------- static pools (whole-kernel lifetime) ------------
        kvq = tc.alloc_tile_pool(name="kvq", bufs=1, side="right")
        k_nope = [[kvq.tile([P, 512], BF, name=f"kn{h}_{b}", tag=f"kn{h}_{b}")
                   for b in range(NB)] for h in range(HPC)]
        k_rope = [[kvq.tile([P, 512], BF, name=f"krp{p}_{b}", tag=f"krp{p}_{b}")
                   for b in range(NB)] for p in range(HPC // 2)]
        q_nope = [[kvq.tile([P, 512], BF, name=f"qn{h}_{b}", tag=f"qn{h}_{b}")
                   for b in range(NB)] for h in range(HPC)]
        q_rope = [[kvq.tile([P, 512], BF, name=f"qrp{p}_{b}", tag=f"qrp{p}_{b}")
                   for b in range(NB)] for p in range(HPC // 2)]
        v4 = [kvq.tile([P, HPC * DV], BF, name=f"v{t}", tag=f"v{t}")
              for t in range(KT)]

        latp = tc.alloc_tile_pool(name="latp", bufs=1)
        c_kv = [latp.tile([P, S], BF, name=f"ckv{m}", tag=f"ckv{m}")
                for m in range(KVT)]
        q_lat = [latp.tile([P, S], BF, name=f"ql{m}", tag=f"ql{m}")
                 for m in range(QLT)]

        upw = tc.alloc_tile_pool(name="upw", bufs=1)
        wuk = upw.tile([P, KVT * (DN + DR + DV) * HPC], BF, name="wuk",
                       tag="wuk")
        wuq = upw.tile([P, QLT * (DN + DR) * HPC], BF, name="wuq", tag="wuq")
        cs_t = upw.tile([P, 2 * S], BF, name="cs_t", tag="cs_t")
        ku = [wuk[:, k * 512:(k + 1) * 512] for k in range(KVT)]
        kr = [wuk[:, 2048 + k * 256:2048 + (k + 1) * 256] for k in range(KVT)]
        vu = [wuk[:, 3072 + k * 512:3072 + (k + 1) * 512] for k in range(KVT)]
        wu = [wuq[:, k * 512:(k + 1) * 512] for k in range(QLT)]
        wr = [wuq[:, 3072 + k * 256:3072 + (k + 1) * 256] for k in range(QLT)]
        cob = [cs_t[:, b * 512:(b + 1) * 512] for b in range(NB)]
        snb = [cs_t[:, S + b * 512:S + (b + 1) * 512] for b in range(NB)]

        msc = tc.alloc_tile_pool(name="msc", bufs=1)
        ones_k = msc.tile([P, P], BF, name="ones_k", tag="ones_k")
        nc.vector.memset(ones_k[:], 1.0)
        if variant == "causal":
            m01 = msc.tile([P, 4 * 512], BF, name="m01", tag="m01")

        # sharded down weights (single packed DMA, sync queue, needed first)
        swp = tc.alloc_tile_pool(name="swp", bufs=1)
        wds = swp.tile([P, KT * SH], BF, name="wds", tag="wds")
        wds_chunks = [(0, 2), (2, 2), (4, 6), (10, 6)]
        for c, (k0, nk) in enumerate(wds_chunks):
            q = nc.sync if c % 2 == 0 else nc.scalar
            q.dma_start(out=wds[:, k0 * SH:(k0 + nk) * SH],
                        in_=wds_d[:, k0 * SH:(k0 + nk) * SH])
        wks = [wds[:, k * SH:k * SH + P] for k in range(KT)]
        wqs = [wds[:, k * SH + P:(k + 1) * SH] for k in range(KT)]

        # ---------------- front: pipelined downs + gathers ----------------
        dgp = tc.alloc_tile_pool(name="dgather", bufs=1, space="DRAM")
        gin_a = dgp.tile([SH, 512], BF, name="gin_a", tag="gin_a")
        gout_a = dgp.tile([4 * SH, 512], BF, name="gout_a", tag="gout_a")
        gin_b = dgp.tile([SH, 1024], BF, name="gin_b", tag="gin_b")
        gout_b = dgp.tile([4 * SH, 1024], BF, name="gout_b", tag="gout_b")

        hidp = tc.alloc_tile_pool(name="hidp", bufs=1)
        hb = [hidp.tile([P, 512], BF, name=f"hb{k}", tag=f"hb{k}")
              for k in range(KT)]
        dwp = tc.alloc_tile_pool(name="dwp", bufs=2)
        stp = tc.alloc_tile_pool(name="stp", bufs=2)

        def load_hid(n):
            for k in range(KT):
                q = nc.sync if k % 2 == 0 else nc.scalar
                q.dma_start(out=hb[k][:],
                            in_=hidT[k * P:(k + 1) * P,
                                     n * 512:(n + 1) * 512])

        def reload(gout, c_lo, width):
            # gpsimd queue: parks on the CC completion semaphore without
            # blocking the sync/scalar load streams
            for r in range(4):
                nc.gpsimd.dma_start(
                    out=c_kv[r][:, c_lo:c_lo + width],
                    in_=gout[SH * r:SH * r + P, 0:width])
                d0 = QSH * r
                row = SH * r + P
                left = QSH
                while left:
                    t, p0 = d0 // P, d0 % P
                    take = min(P - p0, left)
                    nc.gpsimd.dma_start(
                        out=q_lat[t][p0:p0 + take, c_lo:c_lo + width],
                        in_=gout[row:row + take, 0:width])
                    row += take
                    d0 += take
                    left -= take

        def partial_down(n):
            # 1/4-sharded c_kv + q_lat for column block n -> bounce buffer
            tg = 0 if n % 2 else 3
            pk = fps.tile([P, 512], FP, name="pdk", tag=f"rp{tg}")
            p0 = fps.tile([P, 512], FP, name="pd0", tag=f"rp{tg + 1}")
            p1 = fps.tile([64, 512], FP, name="pd1", tag=f"rp{tg + 2}")
            for k in range(KT):
                st, sp = (k == 0), (k == KT - 1)
                mm(pk[:], wks[k][:], hb[k][:], start=st, stop=sp)
                mm(p0[:], wqs[k][:, 0:P], hb[k][:], start=st, stop=sp)
                mm(p1[:], wqs[k][:, P:QSH], hb[k][:], start=st, stop=sp)
            ck = stp.tile([P, 512], BF, name="stk", tag="stk")
            q0 = stp.tile([P, 512], BF, name="st0", tag="st0")
            q1 = stp.tile([64, 512], BF, name="st1", tag="st1")
            nc.scalar.copy(ck[:], pk[:])
            nc.scalar.activation(q0[:], p0[:],
                                 mybir.ActivationFunctionType.Copy,
                                 scale=float(SCALE))
            nc.scalar.activation(q1[:], p1[:],
                                 mybir.ActivationFunctionType.Copy,
                                 scale=float(SCALE))
            if n == 1:
                gi, c0 = gin_a, 0
            else:
                gi, c0 = gin_b, (n - 2) * 512
            # scalar queue: keeps the sync queue a pure load stream
            nc.scalar.dma_start(out=gi[0:P, c0:c0 + 512], in_=ck[:])
            nc.scalar.dma_start(out=gi[P:2 * P, c0:c0 + 512], in_=q0[:])
            nc.scalar.dma_start(out=gi[2 * P:SH, c0:c0 + 512], in_=q1[:])

        # One shared 6-bank PSUM pool for all front matmuls: partials use
        # rp0-2, repl-ckv rp0-3, repl-ql rp0-5 (WAR-serialized, sequential
        # phases anyway).
        fps = tc.alloc_tile_pool(name="fps", bufs=1, space="PSUM")

        # Order: partial(1) + CC-A first (the CC bootstrap barrier takes
        # ~65us, so CC-A starts the moment it ends), partials 2-3 + CC-B
        # next (CC-B runs right behind CC-A on the CC engine), then the
        # replicated block-0 downs fill the remaining gather window. Block-0
        # hid tiles load last so the hb tile rotation leaves them resident
        # for both repl passes.
        load_hid(1)
        partial_down(1)
        nc.gpsimd.collective_compute(
            "AllGather", mybir.AluOpType.bypass, replica_groups=REP,
            ins=[gin_a[:].opt()], outs=[gout_a[:].opt()])

        load_hid(2)
        partial_down(2)
        load_hid(3)
        partial_down(3)
        nc.gpsimd.collective_compute(
            "AllGather", mybir.AluOpType.bypass, replica_groups=REP,
            ins=[gin_b[:].opt()], outs=[gout_b[:].opt()])
        reload(gout_a, 512, 512)

        load_hid(0)
        # prefetch ALL ql-down weight chunks upfront (8 x 2-ktile chunks,
        # fully resident): their 3MB must transfer before CC-B steals the
        # DMA bandwidth, else the repl-ql pass starves mid-stream
        wq_ch = [dwp.tile([P, 2 * QL], BF, name=f"wqfc{c}", tag=f"wqfc{c}",
                          bufs=1) for c in range(8)]
        for c in range(8):
            q = nc.sync if c % 2 == 0 else nc.scalar
            q.dma_start(out=wq_ch[c][:],
                        in_=wqf_d[:, c * 2 * QL:(c + 1) * 2 * QL])
        pk4 = [fps.tile([P, 512], FP, name=f"rpk{m}", tag=f"rp{m}")
               for m in range(KVT)]
        for c in range(8):  # stream 2-ktile chunks of full kv-down weights
            wt = dwp.tile([P, 2 * KVL], BF, name="wkfc", tag="wkfc")
            q = nc.sync if c % 2 == 0 else nc.scalar
            q.dma_start(out=wt[:],
                        in_=wkf_d[:, c * 2 * KVL:(c + 1) * 2 * KVL])
            for kk in range(2):
                k = 2 * c + kk
                for m in range(KVT):
                    mm(pk4[m][:], wt[:, kk * KVL + m * P:kk * KVL + (m + 1) * P],
                       hb[k][:], start=(k == 0), stop=(k == KT - 1))
        for m in range(KVT):
            if m % 2 == 0:
                nc.scalar.copy(c_kv[m][:, 0:512], pk4[m][:])
            else:
                nc.vector.tensor_copy(c_kv[m][:, 0:512], pk4[m][:])
        pq6 = [fps.tile([P, 512], FP, name=f"rpq{m}", tag=f"rp{m}")
               for m in range(QLT)]
        for c in range(8):
            wt = wq_ch[c]
            for kk in range(2):
                k = 2 * c + kk
                for m in range(QLT):
                    mm(pq6[m][:], wt[:, kk * QL + m * P:kk * QL + (m + 1) * P],
                       hb[k][:], start=(k == 0), stop=(k == KT - 1))
        for m in range(QLT):
            nc.scalar.activation(q_lat[m][:, 0:512], pq6[m][:],
                                 mybir.ActivationFunctionType.Copy,
                                 scale=float(SCALE))
        # up-proj weights etc. at the TAIL of the sync/scalar load streams:
        # they arrive by ~80us (needed ~105us) without contending with the
        # critical hid0/wkf/wqf transfers in the 35-70us window
        nc.sync.dma_start(out=wuk[:], in_=wuk_d[:, :])
        nc.scalar.dma_start(out=cs_t[:], in_=cs_d[:, :])
        nc.sync.dma_start(out=wuq[:], in_=wuq_d[:, :])
        if variant == "causal":
            nc.scalar.dma_start(out=m01[:], in_=m01_d[:, :])
        reload(gout_b, 1024, 1024)
        fps.release()

        stp.release()
        dwp.release()
        hidp.release()
        swp.release()

        # ---------------- per-q-block pipeline ----------------------------
        wop = tc.alloc_tile_pool(name="wop", bufs=1)
        wot = wop.tile([P, HPC * HID], BF, name="wot", tag="wot")
        nc.gpsimd.dma_start(out=wot[:], in_=wo_d[:, :])
        wo = [wot[:, k * HID:(k + 1) * HID] for k in range(HPC)]

        with tc.tile_pool(name="tp2", bufs=2) as tp, \
             tc.tile_pool(name="probs", bufs=5) as prp, \
             tc.tile_pool(name="dnt", bufs=8) as dnp, \
             tc.tile_pool(name="attn", bufs=5) as atp, \
             tc.tile_pool(name="osb", bufs=2) as osp, \
             tc.tile_pool(name="rdn", bufs=2) as rdp, \
             tc.tile_pool(name="rbp", bufs=2) as rbp, \
             tc.tile_pool(name="ps_s", bufs=3, space="PSUM") as ps_s, \
             tc.tile_pool(name="ps_pv", bufs=2, space="PSUM") as ps_pv, \
             tc.tile_pool(name="ps_den", bufs=1, space="PSUM") as ps_den, \
             tc.tile_pool(name="ps_uo", bufs=2, space="PSUM") as pso:

            def up_block(wt, kt, src, out_tile, lo, hi, j, ev):
                ps = pso.tile([P, 512], FP, name="psu", tag="uo")
                for k in range(kt):
                    mm(ps[:], wt[k][:, lo:hi],
                       src[k][:, j * 512:(j + 1) * 512],
                       start=(k == 0), stop=(k == kt - 1))
                if ev == 0:
                    nc.scalar.copy(out_tile[:], ps[:])
                else:
                    nc.vector.tensor_copy(out_tile[:], ps[:])

            def rope_block(x, j):
                # in-place rope on pair-packed [128,512] tile x (block j)
                t2 = tp.tile([P, 512], BF, name="t2", tag="t2")
                for q in range(4):
                    src = (q // 2) * 64 + (32 if q % 2 == 0 else 0)
                    nc.vector.tensor_copy(t2[q * 32:(q + 1) * 32],
                                          x[src:src + 32])
                nc.vector.tensor_tensor(t2[:], t2[:], snb[j],
                                        mybir.AluOpType.mult)
                nc.vector.tensor_tensor(x[:], x[:], cob[j],
                                        mybir.AluOpType.mult)
                nc.vector.tensor_tensor(x[:], x[:], t2[:],
                                        mybir.AluOpType.add)

            def U2(j):
                # rope tiles first: their DVE chains drain behind the nope
                # matmuls before att(j)'s first score group reads them
                for p in range(HPC // 2):
                    up_block(kr, KVT, c_kv, k_rope[p][j],
                             p * 2 * DR, (p + 1) * 2 * DR, j, p % 2)
                    rope_block(k_rope[p][j], j)
                for h in range(HPC):
                    up_block(ku, KVT, c_kv, k_nope[h][j],
                             h * DN, (h + 1) * DN, j, h % 2)
                for t in range(4 * j, 4 * j + 4):
                    ps = pso.tile([P, HPC * DV], FP, name="psv", tag="uo")
                    for k in range(KVT):
                        mm(ps[:], c_kv[k][:, t * P:(t + 1) * P], vu[k][:],
                           start=(k == 0), stop=(k == KVT - 1))
                    if t % 2 == 0:
                        nc.scalar.copy(v4[t][:], ps[:])
                    else:
                        nc.vector.tensor_copy(v4[t][:], ps[:])

            def U1(j):
                for p in range(HPC // 2):
                    up_block(wr, QLT, q_lat, q_rope[p][j],
                             p * 2 * DR, (p + 1) * 2 * DR, j, p % 2)
                    rope_block(q_rope[p][j], j)
                for h in range(HPC):
                    up_block(wu, QLT, q_lat, q_nope[h][j],
                             h * DN, (h + 1) * DN, j, h % 2)

            def o_units(j, attn_sb):
                # one unit = 4 matmuls + evict + chunk write for (t, nn);
                # units are sprinkled between att(j+1)'s score/pv pairs as
                # pure-PE filler that covers the exp->mask->pv chains
                obs = {}

                def unit(t, nn):
                    def emit():
                        if nn == 0:
                            obs[t] = osp.tile([P, HID], BF, name="ob",
                                              tag="ob")
                        ob = obs[t]
                        po = pso.tile([P, 512], FP, name="po", tag="uo")
                        for kk in range(HPC):
                            mm(po[:], attn_sb[kk][:, t * P:(t + 1) * P],
                               wo[kk][:, nn * 512:(nn + 1) * 512],
                               start=(kk == 0), stop=(kk == HPC - 1))
                        if nn % 2 == 0:
                            nc.scalar.copy(ob[:, nn * 512:(nn + 1) * 512],
                                           po[:])
                        else:
                            nc.vector.tensor_copy(
                                ob[:, nn * 512:(nn + 1) * 512], po[:])
                        nc.gpsimd.dma_start(
                            out=o_out[(j * 4 + t) * P:(j * 4 + t + 1) * P,
                                      nn * 512:(nn + 1) * 512],
                            in_=ob[:, nn * 512:(nn + 1) * 512])
                    return emit

                return [unit(t, nn) for t in range(4) for nn in range(NB)]

            def o_proj(j, attn_sb):
                for u in o_units(j, attn_sb):
                    u()

            def dve_or_pool(level=1):
                return nc.vector

            def tail(j, h, pv, den_acc, attn_sb):
                # ones lhsT is [128,128], so the den matmul replicates the
                # denominator across all partitions: no partition_broadcast
                den = ps_den.tile([P, 512], FP, name="den", tag="den")
                mm(den[:], ones_k[:], den_acc[:], start=True, stop=True)
                rb = rbp.tile([P, 512], FP, name="rb", tag="rb")
                nc.vector.reciprocal_approx_fast(out=rb[:], in_=den[:])
                at = atp.tile([P, 512], BF, name="at", tag="at")
                nc.vector.tensor_tensor(at[:], pv[:], rb[:],
                                        mybir.AluOpType.mult)
                attn_sb.append(at)

            def att(j, attn_sb, deferred, ounits=()):
                ounits = list(ounits)
                nkt = nkt_of(j)
                if variant == "causal":
                    # interleave diagonal (masked, longer dep chain) tiles
                    # with non-diagonal ones for extra pipeline slack
                    diag = list(range(4 * j, nkt))
                    rest = list(range(4 * j))
                    korder = []
                    while diag or rest:
                        if diag:
                            korder.append(diag.pop(0))
                        if rest:
                            korder.append(rest.pop(0))
                else:
                    korder = list(range(nkt))
                seq = [(h, ki) for h in range(HPC) for ki in korder]
                pos_of = {t: i for i, t in enumerate(seq)}
                emitted = {}
                cursor = [0]

                def emit_ss(h, ki):
                    pp_, hh = h // 2, (h % 2) * DR
                    b, kc = ki // 4, (ki % 4) * P
                    ss = ps_s.tile([P, 512], FP, name="ss", tag="ss")
                    mm(ss[:], k_nope[h][b][:, kc:kc + P], q_nope[h][j][:],
                       start=True, stop=False)
                    mm(ss[:], k_rope[pp_][b][hh:hh + DR, kc:kc + P],
                       q_rope[pp_][j][hh:hh + DR], start=False, stop=True)
                    return ss

                def ensure_ss(idx):
                    while cursor[0] <= min(idx, len(seq) - 1):
                        t = seq[cursor[0]]
                        emitted[t] = emit_ss(*t)
                        cursor[0] += 1

                for h in range(HPC):
                    pv = ps_pv.tile([P, 512], FP, name="pv", tag="pv")
                    stack = []  # (level, tile) pairwise den reduction
                    flush_at = 5 if nkt > 5 else 3
                    for i_k, ki in enumerate(korder):
                        if i_k == flush_at and deferred:
                            tail(*deferred.pop())
                        ensure_ss(pos_of[(h, ki)] + 2)
                        ss = emitted.pop((h, ki))
                        pr = prp.tile([P, 512], BF, name="pr", tag="pr")
                        nc.scalar.activation(
                            pr[:], ss[:], mybir.ActivationFunctionType.Exp)
                        off = P * ki - 512 * j
                        if variant == "causal" and off >= 0:
                            v = off // P
                            nc.vector.tensor_tensor(
                                pr[:], pr[:], m01[:, v * 512:(v + 1) * 512],
                                mybir.AluOpType.mult)
                        mm(pv[:], v4[ki][:, h * DV:(h + 1) * DV], pr[:],
                           start=(i_k == 0), stop=(i_k == nkt - 1))
                        if ounits and i_k % 2 == 1:
                            ounits.pop(0)()
                        cur = (0, pr)
                        while stack and stack[-1][0] == cur[0]:
                            l, a = stack.pop()
                            d = dnp.tile([P, 512], BF, name="dt", tag="dt")
                            dve_or_pool(l).tensor_tensor(
                                d[:], a[:], cur[1][:], mybir.AluOpType.add)
                            cur = (l + 1, d)
                        stack.append(cur)
                    while len(stack) > 1:
                        l1, a1 = stack.pop()
                        l0, a0 = stack.pop()
                        d = dnp.tile([P, 512], BF, name="dt", tag="dt")
                        dve_or_pool(1).tensor_tensor(
                            d[:], a0[:], a1[:], mybir.AluOpType.add)
                        stack.append((l1 + 1, d))
                    den_acc = stack[0][1]
                    deferred.append((j, h, pv, den_acc, attn_sb))
                while ounits:
                    ounits.pop(0)()

            prev_sb = None
            deferred = []
            for j in range(NB):
                U2(j)
                U1(j)
                if deferred:
                    tail(*deferred.pop())
                ou = o_units(j - 1, prev_sb) if prev_sb is not None else ()
                attn_sb = []
                att(j, attn_sb, deferred, ou)
                prev_sb = attn_sb
            if deferred:
                tail(*deferred.pop())
            o_proj(NB - 1, prev_sb)

        wop.release()
        msc.release()
        upw.release()
        latp.release()
        kvq.release()

    nc.compile()
    return nc


def _get(variant):
    if variant not in _cache:
        _cache[variant] = _build_v2(variant)
    return _cache[variant]


def _host_prep(inputs):
    hs = np.ascontiguousarray(inputs["hidden_states"], dtype=np.float32)
    mask = np.asarray(inputs["attention_mask"], dtype=np.float32)
    pos = np.asarray(inputs["position_ids"])
    B = hs.shape[0]

    causal = np.where(np.tril(np.ones((S, S), dtype=bool)), np.float32(0.0),
                      np.float32(NEG))
    variant = "causal"
    for b in range(B):
        if not np.array_equal(mask[b, 0], causal):
            variant = "zeros" if not mask.any() else "generic"
            break

    inv_freq = (1.0 / (ROPE_BASE ** (np.arange(0, DR, 2, dtype=np.float32) / DR)))
    css = []
    for b in range(B):
        t = pos[b].astype(np.float32)
        freqs = t[:, None] * inv_freq[None, :]  # [S, 32]
        cf = np.cos(freqs).T  # [32, S]
        sf = np.sin(freqs).T
        cs = np.empty((128, 2 * S), dtype=np.float32)
        for q in range(4):
            cs[q * 32:(q + 1) * 32, :S] = cf
            cs[q * 32:(q + 1) * 32, S:] = sf if q % 2 else -sf
        css.append(np.ascontiguousarray(cs).astype(ml_dtypes.bfloat16))
    return hs, mask, css, variant


def _pack_cols(mats):
    """hstack row-tile k of each matrix: out[:, :] = [m[128k:128k+128,:] ...
    for k] laid out k-major."""
    kt = mats[0].shape[0] // P
    cols = sum(m.shape[1] for m in mats)
    out = np.empty((P, kt * cols), dtype=np.float32)
    for k in range(kt):
        c = 0
        for m in mats:
            w = m.shape[1]
            out[:, k * cols + c:k * cols + c + w] = m[k * P:(k + 1) * P]
            c += w
    return out


def _bf(a):
    return np.ascontiguousarray(a).astype(ml_dtypes.bfloat16)


def kernel(**inputs):
    hs, mask, css, variant = _host_prep(inputs)
    assert variant in ("causal", "zeros"), "generic mask not supported in v2"
    nc = _get(variant)

    w_qd = np.asarray(inputs["W_q_down"], dtype=np.float32)
    w_kvd = np.asarray(inputs["W_kv_down"], dtype=np.float32)
    W_qu = np.asarray(inputs["W_q_up"], dtype=np.float32)
    W_qr = np.asarray(inputs["W_q_rope"], dtype=np.float32)
    W_ku = np.asarray(inputs["W_k_up"], dtype=np.float32)
    W_kr = np.asarray(inputs["W_k_rope"], dtype=np.float32)
    W_vu = np.asarray(inputs["W_v_up"], dtype=np.float32)
    W_o = np.asarray(inputs["W_o"], dtype=np.float32)

    hidT = [np.ascontiguousarray(hs[b].T).astype(ml_dtypes.bfloat16)
            for b in range(2)]
    wkf_p = _bf(_pack_cols([w_kvd]))
    wqf_p = _bf(_pack_cols([w_qd]))
    # m01[p, 512v + f] = 1 where query f can see key (128v + p), else 0
    f = np.arange(512)[None, None, :]
    kg = np.arange(4)[:, None, None] * 128 + np.arange(P)[None, :, None]
    m01 = (f >= kg).astype(np.float32).transpose(1, 0, 2).reshape(P, 4 * 512)
    m01 = np.ascontiguousarray(m01).astype(ml_dtypes.bfloat16)

    in_maps = []
    for core in range(NCORES):
        b, hg = divmod(core, NCORES // 2)
        qsl = slice(hg * HPC * DN, (hg + 1) * HPC * DN)
        rsl = slice(hg * HPC * DR, (hg + 1) * HPC * DR)
        vsl = slice(hg * HPC * DV, (hg + 1) * HPC * DV)
        # wuk layout: [ku k-tiles | kr k-tiles | vu k-tiles]
        wuk_p = np.empty((P, KVT * 1280), dtype=np.float32)
        for k in range(KVT):
            wuk_p[:, k * 512:(k + 1) * 512] = W_ku[k * P:(k + 1) * P, qsl]
            wuk_p[:, 2048 + k * 256:2048 + (k + 1) * 256] = \
                W_kr[k * P:(k + 1) * P, rsl]
            wuk_p[:, 3072 + k * 512:3072 + (k + 1) * 512] = \
                W_vu[k * P:(k + 1) * P, vsl]
        wuq_p = np.empty((P, QLT * 768), dtype=np.float32)
        for k in range(QLT):
            wuq_p[:, k * 512:(k + 1) * 512] = W_qu[k * P:(k + 1) * P, qsl]
            wuq_p[:, 3072 + k * 256:3072 + (k + 1) * 256] = \
                W_qr[k * P:(k + 1) * P, rsl]
        wo_p = np.empty((P, HPC * HID), dtype=np.float32)
        for k in range(HPC):
            wo_p[:, k * HID:(k + 1) * HID] = \
                W_o[hg * HPC * DV + k * P:hg * HPC * DV + (k + 1) * P, :]
        m = {
            "hidT": hidT[b],
            "wds": _bf(_pack_cols([w_kvd[:, hg * P:(hg + 1) * P],
                                   w_qd[:, hg * QSH:(hg + 1) * QSH]])),
            "wkf": wkf_p,
            "wqf": wqf_p,
            "wuk": _bf(wuk_p),
            "wuq": _bf(wuq_p),
            "wop": _bf(wo_p),
            "cs": css[b],
        }
        if variant == "causal":
            m["m01"] = m01
        in_maps.append(m)

    global _last_in_maps, _last_nc
    _last_in_maps, _last_nc = in_maps, nc
    res = run_bass_kernel_spmd(nc, in_maps, core_ids=list(range(NCORES)))
    out = np.zeros((2, S, HID), dtype=np.float32)
    for core in range(NCORES):
        b = core // (NCORES // 2)
        out[b] += res.results[core]["o"].astype(np.float32)
    return out
